# revision 18
# baseline (speedup 1.0000x reference)
"""Trainium2 Bass kernel for nn_CritiGraph.

Contract: kernel(**inputs) takes FULL unsharded inputs (numpy arrays, keyed as
reference.setup_inputs()), shards the station dim T=1024 across 8 NeuronCores
(128 stations per core; station = SBUF partition), runs one SPMD Bass kernel,
and returns (selected_locs [1024,8] int32, real_loss [1024] float32).

Math notes (validated bit-level against the jax reference):
  cos_sim(a,b) = sg * (1 - e/16), e = bitlen((|a|^|b|)+1)
               = sg * (142 - eb)/16, eb = biased exponent of float32(x+1)
  Negated candidates (-res) share |.| with res, so the big loop runs over 33
  candidates (32 flips + original), not 65; only signs differ.
  Per station t, dim p, candidate c:
    loss[c,p] = K0[p] + sa_half[c,p]*(K1h[p] + acc1[c,p]) + acc2[c,p] + base...
  where acc1 = sum_s hh[s,p]*eb[s,c,p],  hh = -(1/32)*w*A*sgn_b
        acc2 = sum_s wq[s]*(142-eb)^2,   wq = w/16384
        K1h  = -142 * sum_s hh,          K0 = sum_s w*A^2
        A    = (cos_sn_sum - cos_sn)/8 - eu,  w = mask*|eu|/lth
"""
import os
from contextlib import ExitStack

import numpy as np

import concourse.bass as bass
import concourse.bacc as bacc
import concourse.mybir as mybir
import concourse.tile as tile
from concourse.bass_utils import run_bass_kernel_spmd

Alu = mybir.AluOpType
Act = mybir.ActivationFunctionType
dt = mybir.dt

P = 128          # stations per core (= partitions)
H = 16
TP = 8
S = 64
D = 256
C2 = 33          # candidates in the halved domain (32 flips + original)
CP = C2 * TP     # 264
NC_CORES = 8
SB = 4           # neighbors per big-loop chunk
N_CHUNK = S // SB


def _bcast(ap, axis, shape):
    return ap.unsqueeze(axis).broadcast_to(shape)


def _tt(nc, out, in0, in1, op):
    """Plain tensor_tensor; Bacc.compile() splits any multi-wait instruction
    into legal single-wait form, so no ISA workarounds are needed here."""
    nc.vector.tensor_tensor(out, in0, in1, op)




def _iabs(nc, out, in_, zero_i, scratch):
    """|x| for int32: abs_max-with-imm fails the ISA check, so 0-x then max."""
    n = in_.shape[1]
    nc.vector.tensor_tensor(scratch[:, 0:n],
                            zero_i[:, 0:1].broadcast_to([in_.shape[0], n]),
                            in_, Alu.subtract)
    nc.vector.tensor_tensor(out, in_, scratch[:, 0:n], Alu.max)


def critigraph_body(ctx: ExitStack, tc, outs, ins):
    nc = tc.nc
    (sta_d, nei_d, rand_d, semb_d, nemb_d, mask_d, rv_d, tr_d, ci_d, cf_d) = ins
    sel_d, rl_d = outs

    f32, i32, u32 = dt.float32, dt.int32, dt.uint32

    const = ctx.enter_context(tc.tile_pool(name="const", bufs=1))
    small = ctx.enter_context(tc.tile_pool(name="small", bufs=1))
    junk = ctx.enter_context(tc.tile_pool(name="junk", bufs=4))
    bigp = ctx.enter_context(tc.tile_pool(name="bigp", bufs=2))

    # ---------- loads ----------
    sta = const.tile([P, TP], i32)
    nc.sync.dma_start(sta[:], sta_d[:])
    nei = const.tile([P, S * TP], i32)
    nc.sync.dma_start(nei[:], nei_d[:])
    rand = const.tile([P, H * 2 * TP], i32)
    nc.sync.dma_start(rand[:], rand_d[:])
    semb = const.tile([P, D], f32)
    nc.sync.dma_start(semb[:], semb_d[:])
    maskt = const.tile([P, S], f32)
    nc.sync.dma_start(maskt[:], mask_d[:])
    rv = const.tile([P, TP], f32)
    nc.sync.dma_start(rv[:], rv_d[:])
    tr = const.tile([P, 1], f32)
    nc.sync.dma_start(tr[:], tr_d[:])
    ci = const.tile([P, 512], i32)    # bits_kp[256], bitsm1_kp[256]
    nc.sync.dma_start(ci[:], ci_d[:])
    cf = const.tile([P, 129], f32)    # jlt[64], iota65[65]
    nc.sync.dma_start(cf[:], cf_d[:])
    bits_kp = ci[:, 0:256]
    bitsm1_kp = ci[:, 256:512]
    jlt = cf[:, 0:64]
    iota65 = cf[:, 64:129]

    b142 = const.tile([P, 1], f32)
    nc.vector.memset(b142[:], 142.0)
    zero_i = const.tile([P, 1], i32)
    nc.vector.memset(zero_i[:], 0)
    # DVE-produced copy of jlt: a DVE op must not mix an engine-sem wait with
    # a DMA-sem wait (codegen limit), so don't feed DMA-fresh consts into DVE
    # ops that also depend on engine-produced tiles.
    jltc = const.tile([P, 64], f32)
    nc.vector.tensor_copy(jltc[:], jlt)

    # ---------- routing ----------
    lt = const.tile([P, 64], f32)
    _tt(nc, 
        lt[:].rearrange("t (i j) -> t i j", i=8),
        _bcast(rv[:], 1, [P, 8, 8]),
        _bcast(rv[:], 2, [P, 8, 8]),
        Alu.is_lt)
    eqm = const.tile([P, 64], f32)
    _tt(nc, 
        eqm[:].rearrange("t (i j) -> t i j", i=8),
        _bcast(rv[:], 1, [P, 8, 8]),
        _bcast(rv[:], 2, [P, 8, 8]),
        Alu.is_equal)
    _tt(nc, eqm[:], eqm[:], jltc[:], Alu.mult)
    _tt(nc, lt[:], lt[:], eqm[:], Alu.add)
    rank = const.tile([P, TP], f32)
    nc.vector.tensor_reduce(rank[:], lt[:].rearrange("t (i j) -> t i j", i=8),
                            mybir.AxisListType.X, Alu.add)
    flag = const.tile([P, TP], f32)
    nc.vector.tensor_scalar(flag[:], rank[:], 4.0, None, Alu.is_lt)
    s01 = const.tile([P, 1], f32)
    nc.vector.tensor_scalar(s01[:], tr[:], 0.8, None, Alu.is_lt)
    nc.vector.tensor_scalar(flag[:], flag[:], s01[:, 0:1], None, Alu.mult)


    # ---------- candidate generation (local c: 0..31 flips, 32 = original) ----------
    # flip2[(h k),p] = sta[p] ^ bits[h]  (bits pre-broadcast on host to (h,k,p))
    flip2 = small.tile([P, 256], i32)
    _tt(nc, 
        flip2[:].rearrange("t (hk p) -> t hk p", hk=32),
        sta[:].unsqueeze(1).broadcast_to([P, 32, TP]),
        bits_kp.rearrange("t (hk p) -> t hk p", hk=32),
        Alu.bitwise_xor)
    maskv = small.tile([P, H * 2 * TP], i32)
    _tt(nc, maskv[:], rand[:], bitsm1_kp, Alu.bitwise_and)
    cnc = small.tile([P, CP], i32)
    _tt(nc, cnc[:, 0:256], flip2[:], maskv[:], Alu.bitwise_xor)
    nc.vector.tensor_copy(cnc[:, 256:264], sta[:])

    negsc = small.tile([P, S * TP], i32)
    acnc = small.tile([P, CP], i32)
    _iabs(nc, acnc[:], cnc[:], zero_i, negsc)
    sa_half = small.tile([P, CP], f32)
    nc.vector.tensor_scalar(sa_half[:], cnc[:], 0, None, Alu.is_ge)
    nc.vector.tensor_scalar(sa_half[:], sa_half[:], 0.5, None, Alu.subtract)
    sn_half = small.tile([P, 256], f32)
    nc.vector.tensor_scalar(sn_half[:], cnc[:, 0:256], 0, None, Alu.is_le)
    nc.vector.tensor_scalar(sn_half[:], sn_half[:], 0.5, None, Alu.subtract)

    anei = small.tile([P, S * TP], i32)
    _iabs(nc, anei[:], nei[:], zero_i, negsc)
    sgn_b = small.tile([P, S * TP], f32)
    nc.vector.tensor_scalar(sgn_b[:], nei[:], 0, None, Alu.is_ge)
    nc.vector.tensor_scalar(sgn_b[:], sgn_b[:], 2.0, 1.0, Alu.mult, Alu.subtract)
    asta = small.tile([P, TP], i32)
    _iabs(nc, asta[:], sta[:], zero_i, negsc)
    sgn_s = small.tile([P, TP], f32)
    nc.vector.tensor_scalar(sgn_s[:], sta[:], 0, None, Alu.is_ge)
    nc.vector.tensor_scalar(sgn_s[:], sgn_s[:], 2.0, 1.0, Alu.mult, Alu.subtract)

    # ---------- cos_sn (station vs neighbors) ----------
    x0 = small.tile([P, S * TP], i32)
    _tt(nc, 
        x0[:].rearrange("t (s p) -> t s p", s=S),
        _bcast(asta[:], 1, [P, S, TP]),
        anei[:].rearrange("t (s p) -> t s p", s=S),
        Alu.bitwise_xor)
    f0 = small.tile([P, S * TP], f32)
    nc.scalar.activation(f0[:], x0[:], Act.Copy, bias=1.0)
    eb0 = small.tile([P, S * TP], i32)
    nc.vector.tensor_scalar(eb0[:], f0[:].bitcast(i32), 23, None,
                            Alu.logical_shift_right)
    v0 = small.tile([P, S * TP], f32)
    nc.scalar.activation(v0[:], eb0[:], Act.Copy, bias=142.0 / 16.0,
                         scale=-1.0 / 16.0)
    cos_sn = small.tile([P, S * TP], f32)
    _tt(nc, cos_sn[:], v0[:], sgn_b[:], Alu.mult)
    _tt(nc, 
        cos_sn[:].rearrange("t (s p) -> t s p", s=S),
        cos_sn[:].rearrange("t (s p) -> t s p", s=S),
        _bcast(sgn_s[:], 1, [P, S, TP]),
        Alu.mult)
    cs_sum = small.tile([P, S], f32)
    nc.vector.tensor_reduce(cs_sum[:], cos_sn[:].rearrange("t (s p) -> t s p", s=S),
                            mybir.AxisListType.X, Alu.add)

    # ---------- embeddings: eu ----------
    ns2 = small.tile([P, 1], f32)
    jt = junk.tile([P, D], f32, tag="junk")
    nc.scalar.activation(jt[:], semb[:], Act.Square, accum_out=ns2[:])
    dotv = const.tile([P, S], f32)
    nn2 = const.tile([P, S], f32)
    # single stable scratch per engine: same-engine WAW needs no semaphore,
    # and a never-recycled address avoids the engine+DMA mixed-wait pattern
    jd = const.tile([P, D], f32)
    js = const.tile([P, D], f32)
    # whole nei_emb resident: ONE input DMA, so the 64 dot-product STTs only
    # ever wait on DMA sems (never an engine+DMA mix, which codegen rejects)
    nemb = const.tile([P, S * D], f32)
    nc.sync.dma_start(nemb[:], nemb_d[:])
    for s in range(S):
        nc.vector.scalar_tensor_tensor(
            jd[:], nemb[:, s * D:(s + 1) * D], 1.0, semb[:],
            Alu.bypass, Alu.mult, accum_out=dotv[:, s:s + 1])
        nc.scalar.activation(js[:], nemb[:, s * D:(s + 1) * D], Act.Square,
                             accum_out=nn2[:, s:s + 1])

    prodn = small.tile([P, S], f32)
    nc.vector.tensor_scalar(prodn[:], nn2[:], ns2[:, 0:1], None, Alu.mult)
    sqv = small.tile([P, S], f32)
    nc.scalar.activation(sqv[:], prodn[:], Act.Sqrt)
    inv = small.tile([P, S], f32)
    nc.vector.reciprocal(inv[:], sqv[:])
    eu = small.tile([P, S], f32)
    _tt(nc, eu[:], dotv[:], inv[:], Alu.mult)

    # ---------- weights ----------
    aeu = small.tile([P, S], f32)
    negeu = small.tile([P, S], f32)
    nc.vector.tensor_scalar(negeu[:], eu[:], -1.0, None, Alu.mult)
    _tt(nc, aeu[:], eu[:], negeu[:], Alu.max)
    lth = small.tile([P, 1], f32)
    nc.vector.tensor_reduce(lth[:], maskt[:], mybir.AxisListType.X, Alu.add)
    nc.vector.tensor_scalar(lth[:], lth[:], 1e-12, None, Alu.add)
    linv = small.tile([P, 1], f32)
    nc.vector.reciprocal(linv[:], lth[:])
    wt = small.tile([P, S], f32)
    _tt(nc, wt[:], maskt[:], aeu[:], Alu.mult)
    nc.vector.tensor_scalar(wt[:], wt[:], linv[:, 0:1], None, Alu.mult)
    wq = small.tile([P, S], f32)
    nc.vector.tensor_scalar(wq[:], wt[:], 1.0 / 16384.0, None, Alu.mult)

    # ---------- A, hh, K0, K1h ----------
    A = small.tile([P, S * TP], f32)
    _tt(nc, 
        A[:].rearrange("t (s p) -> t s p", s=S),
        _bcast(cs_sum[:], 2, [P, S, TP]),
        cos_sn[:].rearrange("t (s p) -> t s p", s=S),
        Alu.subtract)
    nc.vector.scalar_tensor_tensor(
        A[:].rearrange("t (s p) -> t s p", s=S),
        A[:].rearrange("t (s p) -> t s p", s=S), 0.125,
        _bcast(eu[:], 2, [P, S, TP]),
        Alu.mult, Alu.subtract)
    hh = small.tile([P, S * TP], f32)
    _tt(nc, hh[:], A[:], sgn_b[:], Alu.mult)
    nc.vector.scalar_tensor_tensor(
        hh[:].rearrange("t (s p) -> t s p", s=S),
        hh[:].rearrange("t (s p) -> t s p", s=S), -1.0 / 32.0,
        _bcast(wt[:], 2, [P, S, TP]),
        Alu.mult, Alu.mult)
    k1h = small.tile([P, TP], f32)
    nc.vector.tensor_reduce(
        k1h[:], hh[:].rearrange("t (s p) -> t s p", s=S).transpose([0, 2, 1]),
        mybir.AxisListType.X, Alu.add)
    nc.vector.tensor_scalar(k1h[:], k1h[:], -142.0, None, Alu.mult)
    a2 = small.tile([P, S * TP], f32)
    nc.scalar.activation(a2[:], A[:], Act.Square)
    _tt(nc, 
        a2[:].rearrange("t (s p) -> t s p", s=S),
        a2[:].rearrange("t (s p) -> t s p", s=S),
        _bcast(wt[:], 2, [P, S, TP]),
        Alu.mult)
    k0 = small.tile([P, TP], f32)
    nc.vector.tensor_reduce(
        k0[:], a2[:].rearrange("t (s p) -> t s p", s=S).transpose([0, 2, 1]),
        mybir.AxisListType.X, Alu.add)

    # ---------- big loop ----------
    acc1 = small.tile([P, CP], f32)
    acc2 = small.tile([P, CP], f32)
    FD = SB * CP
    for k in range(N_CHUNK):
        s0 = k * SB
        xq = bigp.tile([P, FD], i32, tag="xq")
        _tt(nc, 
            xq[:].rearrange("t (s c p) -> t s c p", s=SB, c=C2),
            _bcast(acnc[:].rearrange("t (c p) -> t c p", c=C2), 1, [P, SB, C2, TP]),
            _bcast(anei[:, s0 * TP:(s0 + SB) * TP].rearrange(
                "t (s p) -> t s p", s=SB), 2, [P, SB, C2, TP]),
            Alu.bitwise_xor)
        fq = bigp.tile([P, FD], f32, tag="fq")
        nc.scalar.activation(fq[:], xq[:], Act.Copy, bias=1.0)
        ebq = bigp.tile([P, FD], i32, tag="ebq")
        nc.vector.tensor_scalar(ebq[:], fq[:].bitcast(i32), 23, None,
                                Alu.logical_shift_right)
        # acc1 partial: sum_s hh * eb
        hu = bigp.tile([P, FD], f32, tag="hu")
        _tt(nc, 
            hu[:].rearrange("t (s c p) -> t s c p", s=SB, c=C2),
            ebq[:].rearrange("t (s c p) -> t s c p", s=SB, c=C2),
            _bcast(hh[:, s0 * TP:(s0 + SB) * TP].rearrange(
                "t (s p) -> t s p", s=SB), 2, [P, SB, C2, TP]),
            Alu.mult)
        r1 = acc1 if k == 0 else junk.tile([P, CP], f32, tag="red")
        nc.vector.tensor_reduce(
            r1[:], hu[:].rearrange("t (s c p) -> t s c p", s=SB, c=C2)
            .transpose([0, 2, 3, 1]),
            mybir.AxisListType.X, Alu.add)
        if k > 0:
            _tt(nc, acc1[:], acc1[:], r1[:], Alu.add)
        # acc2 partial: sum_s wq * (142-eb)^2
        uq = bigp.tile([P, FD], f32, tag="uq")
        nc.scalar.activation(uq[:], ebq[:], Act.Square, bias=b142[:, 0:1],
                             scale=-1.0)
        _tt(nc, 
            uq[:].rearrange("t (s c p) -> t s c p", s=SB, c=C2),
            uq[:].rearrange("t (s c p) -> t s c p", s=SB, c=C2),
            wq[:, s0:s0 + SB].unsqueeze(2).unsqueeze(3)
            .broadcast_to([P, SB, C2, TP]),
            Alu.mult)
        r2 = acc2 if k == 0 else junk.tile([P, CP], f32, tag="red")
        nc.vector.tensor_reduce(
            r2[:], uq[:].rearrange("t (s c p) -> t s c p", s=SB, c=C2)
            .transpose([0, 2, 3, 1]),
            mybir.AxisListType.X, Alu.add)
        if k > 0:
            _tt(nc, acc2[:], acc2[:], r2[:], Alu.add)

    # ---------- assemble loss (global c: 0..32 = pos, 33..64 = neg flips) ----------
    base = small.tile([P, CP], f32)
    _tt(nc, 
        base[:].rearrange("t (c p) -> t c p", c=C2),
        acc2[:].rearrange("t (c p) -> t c p", c=C2),
        _bcast(k0[:], 1, [P, C2, TP]),
        Alu.add)
    t1p = small.tile([P, CP], f32)
    _tt(nc, 
        t1p[:].rearrange("t (c p) -> t c p", c=C2),
        acc1[:].rearrange("t (c p) -> t c p", c=C2),
        _bcast(k1h[:], 1, [P, C2, TP]),
        Alu.add)
    l65 = small.tile([P, 65 * TP], f32)
    tmp = small.tile([P, CP], f32)
    _tt(nc, tmp[:], sa_half[:], t1p[:], Alu.mult)
    _tt(nc, l65[:, 0:CP], base[:], tmp[:], Alu.add)
    tmp2 = small.tile([P, 256], f32)
    _tt(nc, tmp2[:], sn_half[:], t1p[:, 0:256], Alu.mult)
    _tt(nc, l65[:, CP:520], base[:, 0:256], tmp2[:], Alu.add)

    nl65 = small.tile([P, 65 * TP], f32)
    nc.vector.tensor_scalar(nl65[:], l65[:], -1.0, None, Alu.mult)

    # ---------- argmin over c per p ----------
    idx_all = small.tile([P, TP], u32)
    for p in range(TP):
        col = nl65[:, p:520:TP]
        mx8 = junk.tile([P, 8], f32, tag="mx8")
        nc.vector.max(mx8[:], col)
        ix8 = junk.tile([P, 8], u32, tag="ix8")
        nc.vector.max_index(ix8[:], mx8[:], col)
        nc.vector.tensor_copy(idx_all[:, p:p + 1], ix8[:, 0:1])
    idxf = small.tile([P, TP], f32)
    nc.vector.tensor_copy(idxf[:], idx_all[:])

    cidx = small.tile([P, TP], f32)
    nc.vector.tensor_scalar(cidx[:], idxf[:], 32.0, None, Alu.subtract)
    _tt(nc, cidx[:], cidx[:], flag[:], Alu.mult)
    nc.vector.tensor_scalar(cidx[:], cidx[:], 32.0, None, Alu.add)

    # ---------- gather outputs via one-hot ----------
    oh = small.tile([P, 65 * TP], f32)
    _tt(nc, 
        oh[:].rearrange("t (c p) -> t c p", c=65),
        _bcast(iota65, 2, [P, 65, TP]),
        _bcast(cidx[:], 1, [P, 65, TP]),
        Alu.is_equal)
    cncf = small.tile([P, 65 * TP], f32)
    nc.vector.tensor_copy(cncf[:, 0:CP], cnc[:])
    nc.vector.tensor_scalar(cncf[:, CP:520], cncf[:, 0:256], -1.0, None, Alu.mult)
    selv = small.tile([P, 65 * TP], f32)
    _tt(nc, selv[:], oh[:], cncf[:], Alu.mult)
    self32 = small.tile([P, TP], f32)
    nc.vector.tensor_reduce(
        self32[:], selv[:].rearrange("t (c p) -> t c p", c=65).transpose([0, 2, 1]),
        mybir.AxisListType.X, Alu.add)
    seli = small.tile([P, TP], i32)
    nc.vector.tensor_copy(seli[:], self32[:])
    nc.sync.dma_start(sel_d[:], seli[:])

    lsel = small.tile([P, 65 * TP], f32)
    _tt(nc, lsel[:], oh[:], l65[:], Alu.mult)
    rsum = small.tile([P, 1], f32)
    nc.vector.tensor_reduce(rsum[:], lsel[:].rearrange("t (c p) -> t c p", c=65),
                            mybir.AxisListType.XY, Alu.add)
    nc.vector.tensor_scalar(rsum[:], rsum[:], 0.125, None, Alu.mult)
    nc.sync.dma_start(rl_d[:], rsum[:])


def make_consts():
    bits = (np.int32(1) << np.arange(16, dtype=np.int32))
    bits_kp = np.broadcast_to(bits[:, None, None], (16, 2, 8)).ravel()
    bitsm1_kp = np.broadcast_to((bits - 1)[:, None, None], (16, 2, 8)).ravel()
    ci = np.concatenate([bits_kp, bitsm1_kp]).astype(np.int32)
    jlt = (np.arange(8)[None, :] < np.arange(8)[:, None]).astype(np.float32).ravel()
    iota65 = np.arange(65, dtype=np.float32)
    cf = np.concatenate([jlt, iota65]).astype(np.float32)
    return (np.broadcast_to(ci, (P, 512)).copy(),
            np.broadcast_to(cf, (P, 129)).copy())


IN_SPECS = [
    ("sta_loc", [P, TP], dt.int32),
    ("nei_loc", [P, S * TP], dt.int32),
    ("rand_numbers", [P, H * 2 * TP], dt.int32),
    ("sta_emb", [P, D], dt.float32),
    ("nei_emb", [P, S * D], dt.float32),
    ("mask", [P, S], dt.float32),
    ("rand_vals", [P, TP], dt.float32),
    ("t_rand", [P, 1], dt.float32),
    ("const_i", [P, 512], dt.int32),
    ("const_f", [P, 129], dt.float32),
]
OUT_SPECS = [
    ("selected", [P, TP], dt.int32),
    ("real_loss", [P, 1], dt.float32),
]

_NC_CACHE = {}


def build_nc():
    if "nc" in _NC_CACHE:
        return _NC_CACHE["nc"]
    nc = bacc.Bacc("TRN2", target_bir_lowering=False, debug=False)
    ins = [nc.dram_tensor(n, s, d, kind="ExternalInput").ap()
           for n, s, d in IN_SPECS]
    outs = [nc.dram_tensor(n, s, d, kind="ExternalOutput").ap()
            for n, s, d in OUT_SPECS]
    with tile.TileContext(nc) as tc:
        with ExitStack() as ctx:
            critigraph_body(ctx, tc, outs, ins)
    # Bacc compile splits multi-wait instructions (TRN2 allows at most one
    # sync wait per instruction) and runs DCE/nop-fusion.
    nc.compile()
    _NC_CACHE["nc"] = nc
    return nc


def shard_inputs(inputs):
    """Full inputs -> list of 8 per-core in_maps."""
    ci, cf = make_consts()
    sta = np.ascontiguousarray(inputs["sta_loc"]).astype(np.int32)
    nei = np.ascontiguousarray(inputs["nei_loc"]).astype(np.int32)
    rnd = np.ascontiguousarray(inputs["rand_numbers"]).astype(np.int32)
    semb = np.ascontiguousarray(inputs["sta_emb"]).astype(np.float32)
    nemb = np.ascontiguousarray(inputs["nei_emb"]).astype(np.float32)
    msk = np.ascontiguousarray(inputs["mask"]).astype(np.float32)
    rv = np.ascontiguousarray(inputs["rand_vals"]).astype(np.float32)
    tr = np.ascontiguousarray(inputs["t_rand"]).astype(np.float32)
    maps = []
    for c in range(NC_CORES):
        sl = slice(c * P, (c + 1) * P)
        maps.append({
            "sta_loc": sta[sl],
            "nei_loc": nei[sl].reshape(P, S * TP),
            "rand_numbers": rnd[sl].reshape(P, H * 2 * TP),
            "sta_emb": semb[sl],
            "nei_emb": nemb[sl].reshape(P, S * D),
            "mask": msk[sl],
            "rand_vals": rv[sl],
            "t_rand": tr[sl].reshape(P, 1),
            "const_i": ci,
            "const_f": cf,
        })
    return maps


def kernel(trace=False, **inputs):
    nc = build_nc()
    in_maps = shard_inputs(inputs)
    res = run_bass_kernel_spmd(nc, in_maps, list(range(NC_CORES)), trace=trace)
    sel = np.concatenate([r["selected"] for r in res.results], axis=0)
    rl = np.concatenate([r["real_loss"] for r in res.results], axis=0)[:, 0]
    out = (sel.astype(np.int32), rl.astype(np.float32))
    if trace:
        return out, res
    return out
